# revision 5
# baseline (speedup 1.0000x reference)
"""Trainium2 Bass kernel for nn_MAMLAwareGANLoss.

Reference computation (B=1024, Z=256, H=W=128, N=H*W=16384):
    fake   = tanh(noise @ Wg)                      # [B, N]
    d_fake = fake @ Wd                             # [B, 1]
    g_loss = mean(softplus(-d_fake))               # (+ 0.0 * sum(d_real) == 0)
    solvability_loss = mean(per-sample flood-fill penalty of (fake == 1.0) walls)
    cur    = mean(fake == 1.0)
    difficulty_loss  = (cur - current_difficulty)^2
    loss   = g_loss + w_s * solvability_loss + w_d * difficulty_loss

Key structural facts used here:
  * real_mazes enters only through `0.0 * sum(d_real)` == exactly 0.0 -> never loaded.
  * "walls" are cells where float32 tanh(x) rounds to exactly 1.0, which requires
    x >= ~9.01.  We prove on the host (Cauchy-Schwarz over the actual inputs:
    max_b ||noise_b|| * max_n ||Wg[:, n]||) that no |x| can exceed the threshold,
    hence wall count == 0 exactly => solvability_loss == 0.0 and cur == 0.0.
    If the bound ever fails we fall back to an exact host recomputation.
  * Therefore the device only computes d_fake = (tanh(noise @ Wg)) @ Wd.

Device sharding (8 cores): shard the N (=H*W) dimension, 2048 columns/core.
Each core computes, for all 1024 samples, the partial dot product
    dpart[b] = sum_{n in shard} tanh((noise @ Wg)[b, n]) * Wd[n]
The host sums the 8 partials, applies softplus and the scalar tail.

All device arithmetic is fp8e4 (e4m3).  Host-side scaling keeps every tensor in
the fp8 normal range: noise*16, Wg*64, Wd*64.  The activation instruction
applies tanh(psum / 1024) (scale folds the 16*64 back out), and the host
divides the partial sums by 64 (Wd scale).  Host-simulated end-to-end rel err
of the fp8 pipeline vs the fp32 reference is ~8e-4 (tolerance 2e-2).

Per-core device program (layout: n on PSUM partitions, b on free axis):
    x[n, b]  = sum_z Wg[z, n] * noiseT[z, b]   one DoubleRow fp8 matmul per
                                               (tile, b-half): both z k-tiles
                                               contracted at 0.5 cycles/row
    t[n, b]  = tanh(x[n, b] / 1024)            (ACT, PSUM->SBUF, fp8 out)
    dpart[b] = sum_n Wd[n] * t[n, b]           DoubleRow fp8 matmuls over tile
                                               pairs, 4 PE column groups
The ACT engine (tanh over 2.1M elems/core at ~1 elem/lane/cycle) is the
critical path; the fp8 matmuls keep the PE well ahead of it.
"""

import numpy as np
import ml_dtypes

B, Z, H, W = 1024, 256, 128, 128
N = H * W               # 16384
NCORES = 8
NSH = N // NCORES       # 2048 columns of Wg per core
P = 128
NT = NSH // P           # 16 n-tiles per core
NB = B                  # 1024 samples (free axis)
BH = NB // 2            # b-half (one matmul / DMA granule)
NPAIR = NT // 2         # 8 tile pairs for the DoubleRow reduction

S_NOISE = 16.0          # host pre-scales (keep fp8 values in normal range)
S_WG = 64.0
S_WD = 64.0
X_SCALE = S_NOISE * S_WG

# float32 tanh(x) rounds to exactly 1.0 only for x >= ~9.01; stay well below.
WALL_SAFE_BOUND = 8.5

_PROG = None  # cached compiled Bass program


def _build_program():
    import concourse.bass as bass
    import concourse.tile as tile
    from concourse import bacc, mybir

    f32 = mybir.dt.float32
    f8 = mybir.dt.float8e4
    Tanh = mybir.ActivationFunctionType.Tanh
    DR = mybir.MatmulPerfMode.DoubleRow

    nc = bacc.Bacc(
        "TRN2", target_bir_lowering=False, debug=False, num_devices=NCORES
    )
    # Host-relaid inputs, partition-major so every DMA is a straight
    # per-partition segment copy:
    #   noise_t : [128(p), 2(z-tile), 2(b-half), 512]   fp8, 256KB
    #   wg_shard: [128(p), 16(tile), 2(z-tile), 128]    fp8, 512KB
    #   wd_shard: [128(p), 16(tile), 1]                 fp8, 2KB
    noise_d = nc.declare_dram_parameter("noise_t", [P, 2, 2, BH], f8, isOutput=False)
    wg_d = nc.declare_dram_parameter("wg_shard", [P, NT, 2, P], f8, isOutput=False)
    wd_d = nc.declare_dram_parameter("wd_shard", [P, NT, 1], f8, isOutput=False)
    # Four partial rows (one per PE column group used by the reduction);
    # the host sums them and divides by S_WD.
    out_d = nc.declare_dram_parameter("dpart", [4, NB], f32, isOutput=True)

    with tile.TileContext(nc) as tc:
        with (
            tc.tile_pool(name="const", bufs=1) as cpool,
            tc.tile_pool(name="t", bufs=8) as tpool,
            tc.tile_pool(name="ps", bufs=3, space="PSUM") as pspool,
            tc.tile_pool(name="dps", bufs=1, space="PSUM") as dpool,
        ):
            # Reduction accumulators (one PSUM bank each): reduce pair p runs
            # in PE column group p%4, accumulating into partition 32*(p%4).
            dd0 = dpool.tile([P, BH], f32, tag="dd0")
            dd1 = dpool.tile([P, BH], f32, tag="dd1")
            dd = [dd0, dd1]

            noise_sb = cpool.tile([P, 2, 2, BH], f8, tag="noise")
            wg_sb = cpool.tile([P, NT, 2, P], f8, tag="wg")
            wd_sb = cpool.tile([P, NT, 1], f8, tag="wd")

            # DMA issue first on every queue.  sync gets noise (gates the
            # first matmuls), gpsimd gets the first wg tiles, vector/scalar
            # the rest.
            nc.sync.dma_start(out=noise_sb[:, :, 0, :], in_=noise_d[:, :, 0, :])
            nc.sync.dma_start(out=noise_sb[:, :, 1, :], in_=noise_d[:, :, 1, :])
            nc.gpsimd.dma_start(out=wg_sb[:, 0:2], in_=wg_d[:, 0:2])
            nc.gpsimd.dma_start(out=wg_sb[:, 2:8], in_=wg_d[:, 2:8])
            nc.gpsimd.dma_start(out=wg_sb[:, 8:16], in_=wg_d[:, 8:16])
            nc.scalar.dma_start(out=wd_sb[:], in_=wd_d[:])

            # --- PE warm-up: keep the tensor engine busy during the DMA wait
            # (HAM unthrottle + p-state ramp).  fp32 matmuls on a memset tile;
            # output lands in dd0, which the real reduction clears via
            # start=True.
            warm_sb = cpool.tile([P, 256], f32, tag="warm")
            nc.vector.memset(warm_sb[:], 0.0)
            # Preload the tanh activation table (~1.3us) during the DMA wait.
            warm_act = cpool.tile([P, 16], f32, tag="warm_act")
            nc.scalar.activation(warm_act[:], warm_sb[:, 0:16], Tanh)
            for _ in range(3):
                nc.tensor.matmul(
                    dd0[0:1, 0:256],
                    warm_sb[:, 0:1],
                    warm_sb[:, 0:256],
                    start=True,
                    stop=True,
                    skip_group_check=True,
                )

            t_pairs = []

            def t_slice(t, h):
                return t_pairs[t // 2][:, t % 2, h * BH : (h + 1) * BH]

            def emit_reduce_quad(g):
                # 4 n-tiles' reductions in 4 distinct PE column groups; they
                # issue back-to-back and run concurrently on the array.
                # (DoubleRow is rejected by walrus at column positions != 0,
                # so these are plain fp8 matmuls.)
                for h in range(2):
                    for j in range(4):
                        t = g * 4 + j
                        nc.tensor.matmul(
                            dd[h][32 * j : 32 * j + 1, :],
                            wd_sb[:, t, :],
                            t_slice(t, h),
                            start=(g == 0),
                            stop=(g == 3),
                            tile_position=(0, 32 * j),
                            skip_group_check=True,
                        )

            for i in range(NT):
                if i % 2 == 0:
                    tp = tpool.tile([P, 2, NB], f8, name=f"t{i // 2}", tag="t")
                    t_pairs.append(tp)
                ps = pspool.tile([P, NB], f32)
                for h in range(2):
                    nc.tensor.matmul(
                        ps[:, h * BH : (h + 1) * BH],
                        wg_sb[:, i],
                        noise_sb[:, :, h, :],
                        start=True,
                        stop=True,
                        perf_mode=DR,
                    )
                nc.scalar.activation(
                    t_pairs[-1][:, i % 2, :], ps[:], Tanh, scale=1.0 / X_SCALE
                )
                # Reduce quads lag the main matmuls so the PE never stalls
                # waiting for a tanh.
                if i % 4 == 3 and i >= 7:
                    emit_reduce_quad(i // 4 - 1)
            emit_reduce_quad(3)

            out_sb = cpool.tile([97, NB], f32, tag="out")
            nc.vector.tensor_copy(out_sb[:, 0:BH], dd0[0:97, :])
            nc.scalar.copy(out_sb[:, BH:NB], dd1[0:97, :])
            nc.sync.dma_start(out=out_d[:], in_=out_sb[0:97:32, :])

    nc.compile()
    return nc


def _get_program():
    global _PROG
    if _PROG is None:
        _PROG = _build_program()
    return _PROG


def _make_in_maps(noise, Wg, Wd):
    f8 = ml_dtypes.float8_e4m3fn
    # noise_t[p, zi, h, c] = fp8(noise[h*512+c, zi*128+p] * 16)
    nq = (noise * S_NOISE).astype(f8)
    noise_t = np.ascontiguousarray(
        nq.T.reshape(2, P, 2, BH).transpose(1, 0, 2, 3)
    )
    in_maps = []
    for c in range(NCORES):
        # wg_t[p, i, zi, cc] = fp8(Wg[zi*128+p, base + i*128+cc] * 64)
        wq = (Wg[:, c * NSH : (c + 1) * NSH] * S_WG).astype(f8)
        wg_t = np.ascontiguousarray(
            wq.reshape(2, P, NT, P).transpose(1, 2, 0, 3)
        )
        # wd_t[p, i, 0] = fp8(Wd[base + i*128 + p] * 64)
        seg = (Wd[c * NSH : (c + 1) * NSH, 0] * S_WD).astype(f8)
        wd_t = np.ascontiguousarray(seg.reshape(NT, P).T.reshape(P, NT, 1))
        in_maps.append({"noise_t": noise_t, "wg_shard": wg_t, "wd_shard": wd_t})
    return in_maps


def run_device(noise, Wg, Wd, trace=False):
    """Run the SPMD kernel on 8 cores; return (d_fake[B] float64, results)."""
    from concourse.bass_utils import run_bass_kernel_spmd

    nc = _get_program()
    in_maps = _make_in_maps(noise, Wg, Wd)
    res = run_bass_kernel_spmd(nc, in_maps, list(range(NCORES)), trace=trace)
    d_fake = np.zeros(NB, np.float64)
    for r in res.results:
        d_fake += np.asarray(r["dpart"], np.float64).reshape(4, NB).sum(axis=0)
    d_fake /= S_WD
    return d_fake, res


def _dilate(v):
    out = v.copy()
    out[:-1, :] |= v[1:, :]
    out[1:, :] |= v[:-1, :]
    out[:, :-1] |= v[:, 1:]
    out[:, 1:] |= v[:, :-1]
    return out


def _host_exact_maze_terms(noise, Wg):
    """Fallback (practically unreachable): exact wall/flood-fill computation."""
    solv = 0.0
    wall_total = 0
    for b0 in range(0, B, 64):
        x = noise[b0 : b0 + 64].astype(np.float32) @ Wg.astype(np.float32)
        fake = np.tanh(x).astype(np.float32)
        for j in range(fake.shape[0]):
            maze = fake[j].reshape(H, W)
            wall = maze == np.float32(1.0)
            nwall = int(wall.sum())
            wall_total += nwall
            pen = 0.0
            if float(wall.mean()) > 0.5:
                pen += 1.0
            if nwall >= 3:
                open_ = ~wall
                visited = np.zeros((H, W), bool)
                visited[1, 1] = True
                while True:
                    nv = visited | (_dilate(visited) & open_)
                    if not (nv & ~visited).any():
                        break
                    visited = nv
                wf = wall.astype(np.float32)
                wa = np.zeros((H, W), np.float32)
                wa[:-1, :] += wf[1:, :]
                wa[1:, :] += wf[:-1, :]
                wa[:, :-1] += wf[:, 1:]
                wa[:, 1:] += wf[:, :-1]
                pen += 0.1 * float((visited & (wa >= 3.0)).sum())
            solv += pen
    solv /= B
    cur = wall_total / float(B * H * W)
    return solv, cur


def kernel(**inputs) -> np.ndarray:
    noise = np.asarray(inputs["noise"], np.float32)
    Wg = np.asarray(inputs["Wg"], np.float32)
    Wd = np.asarray(inputs["Wd"], np.float32)
    p = float(np.asarray(inputs["maml_performance"]).reshape(-1)[0])
    cd = float(np.asarray(inputs["current_difficulty"]).reshape(-1)[0])

    d_fake, _ = run_device(noise, Wg, Wd)

    # g_loss = mean(softplus(-d_fake));  0.0 * sum(d_real) == 0 exactly.
    g_loss = float(np.mean(np.logaddexp(0.0, -d_fake)))

    # Wall existence bound: |x[b,n]| <= max_b||noise_b|| * max_n||Wg[:,n]||.
    rn = float(np.sqrt((noise.astype(np.float64) ** 2).sum(axis=1)).max())
    cn = float(np.sqrt((Wg.astype(np.float64) ** 2).sum(axis=0)).max())
    if rn * cn * 1.0001 < WALL_SAFE_BOUND:
        solv, cur = 0.0, 0.0
    else:  # pragma: no cover - requires |pre-tanh| ~ 28 sigma
        solv, cur = _host_exact_maze_terms(noise, Wg)

    w_s = 0.8 if p < 0.4 else (0.4 if p > 0.6 else 0.6)
    w_d = 0.05 if p < 0.4 else (0.2 if p > 0.6 else 0.1)
    difficulty = (cur - cd) ** 2
    loss = g_loss + w_s * solv + w_d * difficulty
    return np.array(loss, dtype=np.float32)
